# revision 22
# baseline (speedup 1.0000x reference)
"""Causal self-attention Trainium2 kernel — tensor-parallel over heads on 8 NeuronCores.

Problem: B=4, T=2048, C=1024, H=16 heads (head_dim 64), fp32 in/out.
Sharding: 2 heads per core. Each core computes qkv projection for its head
columns, full causal attention for its heads, and a partial output
projection (its W_proj rows); partials are summed on host.

v2: all matmuls in bf16 (fp32r ran ~80% over bf16 speed at N=512 on HW),
softmax exp writes bf16 directly, the softmax denominator is replicated
across 64 PSUM partitions by 64 ones-columns in the AV stationary (one
reciprocal + one multiply per head per q-tile, no scalar-engine copies or
gpsimd broadcasts), and the output projection is DMA'd straight from PSUM
to DRAM with no SBUF bounce.
"""

import numpy as np
from ml_dtypes import bfloat16

import concourse.bass as bass
import concourse.mybir as mybir
from concourse import bacc
from concourse.tile import TileContext
from concourse.masks import make_identity

# Note: walrus's --enable-ldw-opt=true rejects the row-tiled bf16
# LDWEIGHTS this kernel emits ("InstLdweights is not compatible with LDW
# optimization"), so unlike the fp32r variant we leave it at the default.

F32 = mybir.dt.float32
BF16 = mybir.dt.bfloat16

B, T, C, H = 4, 2048, 1024, 16
HD = 64
NCORES = 8
CT = C // 128          # 8 C-tiles (contraction)
QT = 512               # q tile (free dim of S^T matmuls)
KT = 128               # k tile (partition dim of S^T)
VW = 256               # v_sb columns per 128-token subtile: [vA|1s|vB|1s]
SCALE = 1.0 / np.sqrt(HD)

_CACHED = {}


def build_kernel(b=B, t=T, debug_dump=False):
    """Build the per-core SPMD program. t must be a multiple of 512."""
    assert t % QT == 0
    nq = t // QT           # q-tiles per sequence
    nst = t // 128         # 128-token subtiles per sequence
    bt = b * t

    nc = bacc.Bacc("TRN2", target_bir_lowering=False, debug=False,
                   num_devices=NCORES)
    dbg = {}
    if debug_dump:
        dbg["qT"] = nc.dram_tensor("dbg_qT", [128, t], BF16,
                                   kind="ExternalOutput")
        dbg["kT"] = nc.dram_tensor("dbg_kT", [128, t], BF16,
                                   kind="ExternalOutput")
        dbg["v"] = nc.dram_tensor("dbg_v", [128, (t // 128) * VW], BF16,
                                  kind="ExternalOutput")
        dbg["yT"] = nc.dram_tensor("dbg_yT", [128, t], BF16,
                                   kind="ExternalOutput")
        dbg["es"] = nc.dram_tensor("dbg_es", [128, 2 * QT], BF16,
                                   kind="ExternalOutput")
        dbg["rr"] = nc.dram_tensor("dbg_rr", [64, QT], F32,
                                   kind="ExternalOutput")
        dbg["ya"] = nc.dram_tensor("dbg_ya", [128, QT], F32,
                                   kind="ExternalOutput")
        dbg["bc"] = nc.dram_tensor("dbg_bc", [64, QT], F32,
                                   kind="ExternalOutput")

    xT = nc.dram_tensor("xT", [C, bt], BF16, kind="ExternalInput")
    wq = nc.dram_tensor("wq", [C, 128], BF16, kind="ExternalInput")
    wk = nc.dram_tensor("wk", [C, 128], BF16, kind="ExternalInput")
    wv = nc.dram_tensor("wv", [C, 128], BF16, kind="ExternalInput")
    wp = nc.dram_tensor("wp", [128, C], BF16, kind="ExternalInput")
    bq = nc.dram_tensor("bq", [128, 1], F32, kind="ExternalInput")
    bk = nc.dram_tensor("bk", [128, 1], F32, kind="ExternalInput")
    out = nc.dram_tensor("out", [bt, C], BF16, kind="ExternalOutput")

    with TileContext(nc) as tc:
        with (
            tc.tile_pool(name="const", bufs=1) as constp,
            tc.tile_pool(name="xin", bufs=2 * CT) as xin,
            tc.tile_pool(name="qk", bufs=2) as qkp,
            tc.tile_pool(name="yt", bufs=2) as ytp,
            tc.tile_pool(name="es", bufs=3) as esp,
            tc.tile_pool(name="small", bufs=4) as smallp,
            tc.tile_pool(name="outsb", bufs=3) as outp,
            tc.tile_pool(name="ps_s", bufs=2, space="PSUM") as ps_s,
            tc.tile_pool(name="ps_ya", bufs=2, space="PSUM") as ps_ya,
            tc.tile_pool(name="ps_misc", bufs=2, space="PSUM") as ps_misc,
        ):
            # ---- constants / weights ----
            ident = constp.tile([128, 128], F32, tag="ident")
            make_identity(nc, ident[:])
            wq_sb = constp.tile([128, C], BF16, tag="wq")
            wk_sb = constp.tile([128, C], BF16, tag="wk")
            wv_sb = constp.tile([128, C], BF16, tag="wv")
            wp_sb = constp.tile([128, C], BF16, tag="wp")
            # lhsT layout: [p, ct*128 + m] = W[ct*128 + p, m]
            for w_dram, w_sb in ((wq, wq_sb), (wk, wk_sb), (wv, wv_sb)):
                nc.gpsimd.dma_start(
                    out=w_sb[:].rearrange("p (ct m) -> p ct m", ct=CT),
                    in_=w_dram[:].rearrange("(ct p) m -> p ct m", p=128),
                )
            nc.gpsimd.dma_start(out=wp_sb[:], in_=wp[:])
            bq_sb = constp.tile([128, 1], F32, tag="bq")
            bk_sb = constp.tile([128, 1], F32, tag="bk")
            nc.sync.dma_start(out=bq_sb[:], in_=bq[:])
            nc.sync.dma_start(out=bk_sb[:], in_=bk[:])
            one_bf = constp.tile([128, 1], BF16, tag="one")
            nc.vector.memset(one_bf[:], 1.0)

            xts_all = {}

            def load_x(bi):
                tiles = [xin.tile([128, t], BF16, tag="xt",
                                  name=f"xt{bi}_{ct}")
                         for ct in range(CT)]
                for ct in range(CT):
                    nc.gpsimd.dma_start(
                        out=tiles[ct][:],
                        in_=xT[ct * 128:(ct + 1) * 128,
                               bi * t:(bi + 1) * t])
                xts_all[bi] = tiles

            load_x(0)
            for bi in range(b):
                xts = xts_all[bi]
                # v_sb per 128-token subtile: [vA(64)|1s(64)|vB(64)|1s(64)].
                # The ones columns replicate the softmax denominator across
                # PSUM partitions 64..127 of the AV output.
                v_sb = qkp.tile([128, nst * VW], BF16, tag="v")
                v_hw = v_sb[:].rearrange("p (g w) -> p g w", w=128)
                nc.vector.memset(v_hw[:, :, 64:128], 1.0)
                qT_sb = qkp.tile([128, t], BF16, tag="qT")
                kT_sb = qkp.tile([128, t], BF16, tag="kT")
                yT_sb = ytp.tile([128, t], BF16, tag="yT")

                # ================= QKV projection =================
                for colt in range(t // QT):
                    csl = slice(colt * QT, (colt + 1) * QT)
                    for w_sb, dst, bias in (
                        (wk_sb, kT_sb, bk_sb), (wq_sb, qT_sb, bq_sb),
                    ):
                        ps = ps_misc.tile([128, QT], F32, tag="m")
                        for ct in range(CT):
                            nc.tensor.matmul(
                                ps[:],
                                w_sb[:, ct * 128:(ct + 1) * 128],
                                xts[ct][:, csl],
                                start=(ct == 0), stop=(ct == CT - 1),
                            )
                        nc.vector.tensor_scalar_add(
                            out=dst[:, csl], in0=ps[:], scalar1=bias[:])
                    # V^T for this col tile, then transpose to natural layout
                    ps = ps_misc.tile([128, QT], F32, tag="m")
                    for ct in range(CT):
                        nc.tensor.matmul(
                            ps[:], wv_sb[:, ct * 128:(ct + 1) * 128],
                            xts[ct][:, csl],
                            start=(ct == 0), stop=(ct == CT - 1))
                    vt_col = smallp.tile([128, QT], F32, tag="vtcol")
                    nc.vector.tensor_copy(out=vt_col[:], in_=ps[:])
                    tp = ps_misc.tile([128, QT], F32, tag="m")
                    for sj in range(QT // 128):
                        nc.tensor.transpose(
                            tp[:, sj * 128:(sj + 1) * 128],
                            vt_col[:, sj * 128:(sj + 1) * 128],
                            ident[:])
                    for sj in range(QT // 128):
                        st = colt * (QT // 128) + sj
                        src = tp[:, sj * 128:(sj + 1) * 128].rearrange(
                            "p (h w) -> p h w", h=2)
                        dstv = v_sb[:, st * VW:(st + 1) * VW].rearrange(
                            "p (h w) -> p h w", h=2)[:, :, 0:64]
                        nc.vector.tensor_copy(out=dstv, in_=src)

                if bi + 1 < b:
                    load_x(bi + 1)
                if debug_dump and bi == 0:
                    nc.sync.dma_start(out=dbg["qT"][:], in_=qT_sb[:])
                    nc.sync.dma_start(out=dbg["kT"][:], in_=kT_sb[:])
                    nc.sync.dma_start(out=dbg["v"][:], in_=v_sb[:])

                # ================= attention (heads row-paired) ==========
                for qt in range(nq):
                    n_k = (qt + 1) * (QT // KT)   # k-tiles of 128
                    q0 = qt * QT
                    yas = [ps_ya.tile([128, QT], F32, tag="ya",
                                      name=f"ya{_h}")
                           for _h in range(2)]
                    for kt in range(n_k):
                        lo = max(0, kt * KT - q0)
                        # S^T for both heads in one array pass: head A on PE
                        # rows 0-63, head B on rows 64-127 (row tiling).
                        sg = ps_s.tile([128, 2 * QT], F32, tag="sg")
                        es = esp.tile([128, 2 * QT], BF16, tag="es")
                        for h in range(2):
                            hsl = slice(h * 64, (h + 1) * 64)
                            nc.tensor.matmul(
                                sg[:, h * QT + lo:(h + 1) * QT],
                                kT_sb[hsl, kt * KT:(kt + 1) * KT],
                                qT_sb[hsl, q0 + lo:q0 + QT],
                                start=True, stop=True,
                            )
                        # exp for both heads in one op; on diagonal tiles
                        # only the causally-reachable cols [lo:] are computed
                        sg_v = sg[:].rearrange("p (h q) -> p h q", h=2)
                        es_v = es[:].rearrange("p (h q) -> p h q", h=2)
                        nc.scalar.activation(
                            es_v[:, :, lo:], sg_v[:, :, lo:],
                            mybir.ActivationFunctionType.Exp, scale=SCALE)
                        if kt * KT >= q0:
                            # causal band select, both heads in one op
                            nc.gpsimd.affine_select(
                                out=es_v[:, :, lo:lo + KT],
                                in_=es_v[:, :, lo:lo + KT],
                                compare_op=mybir.AluOpType.is_ge,
                                fill=0.0,
                                base=0,
                                channel_multiplier=-1,
                                pattern=[[0, 2], [1, KT]],
                            )
                        if debug_dump and bi == 0 and qt == 0 and kt == 0:
                            nc.sync.dma_start(out=dbg["es"][:], in_=es[:])
                        for h in range(2):
                            nc.tensor.matmul(
                                yas[h][:, lo:QT],
                                v_sb[:, kt * VW + h * 128:
                                     kt * VW + (h + 1) * 128],
                                es[:, h * QT + lo:(h + 1) * QT],
                                start=(kt == 0), stop=(kt == n_k - 1),
                            )
                    for h in range(2):
                        ya = yas[h]
                        # denominator lives replicated on partitions 64-127;
                        # reciprocal there, DMA down to partitions 0-63,
                        # single fused multiply out of PSUM.
                        dn = smallp.tile([128, QT], F32, tag="dn")
                        nc.vector.tensor_copy(out=dn[64:128, :],
                                              in_=ya[64:128, :])
                        dnlo = smallp.tile([64, QT], F32, tag="dnlo")
                        nc.sync.dma_start(out=dnlo[:], in_=dn[64:128, :])
                        bc = smallp.tile([64, QT], F32, tag="bc")
                        nc.vector.reciprocal_approx_fast(
                            out=bc[:], in_=dnlo[:])
                        if debug_dump and bi == 0 and qt == 0 and h == 0:
                            nc.sync.dma_start(out=dbg["rr"][:], in_=bc[:])
                            nc.sync.dma_start(out=dbg["bc"][:], in_=bc[:])
                            yd = smallp.tile([128, QT], F32, tag="yd")
                            nc.vector.tensor_copy(out=yd[0:64, :],
                                                  in_=ya[0:64, :])
                            nc.vector.tensor_copy(out=yd[64:128, :],
                                                  in_=ya[64:128, :])
                            nc.sync.dma_start(out=dbg["ya"][:], in_=yd[:])
                        if h == 0:
                            nc.vector.tensor_mul(
                                out=yT_sb[0:64, q0:q0 + QT],
                                in0=ya[0:64, :], in1=bc[:])
                        else:
                            ytb = smallp.tile([64, QT], BF16, tag="ytb")
                            nc.vector.tensor_mul(
                                out=ytb[:], in0=ya[0:64, :], in1=bc[:])
                            nc.sync.dma_start(
                                out=yT_sb[64:128, q0:q0 + QT], in_=ytb[:])

                    # ======== output projection for this q-tile ========
                    for sj in range(QT // 128):
                        st = qt * (QT // 128) + sj
                        osb = outp.tile([128, C], BF16, tag="osb")
                        for n in range(C // QT):
                            pp = ps_misc.tile([128, QT], F32, tag="m")
                            nc.tensor.matmul(
                                pp[:],
                                yT_sb[:, st * 128:(st + 1) * 128],
                                wp_sb[:, n * QT:(n + 1) * QT],
                                start=True, stop=True)
                            nc.vector.tensor_copy(
                                out=osb[:, n * QT:(n + 1) * QT], in_=pp[:])
                        nc.sync.dma_start(
                            out=out[bi * t + st * 128:
                                    bi * t + (st + 1) * 128, :],
                            in_=osb[:])
                if debug_dump and bi == 0:
                    nc.sync.dma_start(out=dbg["yT"][:], in_=yT_sb[:])

    nc.compile()
    return nc


def _prep_inputs(x, W_attn, b_attn, W_proj, b_proj, b, t):
    xT_full = np.ascontiguousarray(
        x.reshape(b * t, C).T).astype(bfloat16)
    in_maps = []
    for c in range(NCORES):
        sl = slice(c * 128, (c + 1) * 128)
        in_maps.append({
            "xT": xT_full,
            "wq": np.ascontiguousarray(W_attn[:, sl]).astype(bfloat16),
            "wk": np.ascontiguousarray(
                W_attn[:, 1024:2048][:, sl]).astype(bfloat16),
            "wv": np.ascontiguousarray(
                W_attn[:, 2048:3072][:, sl]).astype(bfloat16),
            "wp": np.ascontiguousarray(W_proj[sl, :]).astype(bfloat16),
            "bq": np.ascontiguousarray(b_attn[sl].reshape(128, 1)),
            "bk": np.ascontiguousarray(b_attn[1024:2048][sl].reshape(128, 1)),
        })
    return in_maps


def kernel(x, W_attn, b_attn, W_proj, b_proj, _trace=False):
    from concourse.bass_utils import run_bass_kernel_spmd

    x = np.asarray(x, dtype=np.float32)
    W_attn = np.asarray(W_attn, dtype=np.float32)
    b_attn = np.asarray(b_attn, dtype=np.float32)
    W_proj = np.asarray(W_proj, dtype=np.float32)
    b_proj = np.asarray(b_proj, dtype=np.float32)
    b, t, c = x.shape

    key = (b, t)
    if key not in _CACHED:
        _CACHED[key] = build_kernel(b, t)
    nc = _CACHED[key]

    in_maps = _prep_inputs(x, W_attn, b_attn, W_proj, b_proj, b, t)
    res = run_bass_kernel_spmd(
        nc, in_maps, core_ids=list(range(NCORES)), trace=_trace)

    acc = res.results[0]["out"].astype(np.float32).copy()
    for r in res.results[1:]:
        acc += r["out"]
    acc += b_attn[2048:3072] @ W_proj + b_proj
    out = acc.reshape(b, t, c)
    if _trace:
        kernel.last_result = res
    return out


# revision 25
# speedup vs baseline: 1.0782x; 1.0782x over previous
"""Causal self-attention Trainium2 kernel — tensor-parallel over heads on 8 NeuronCores.

Problem: B=4, T=2048, C=1024, H=16 heads (head_dim 64), fp32 in/out.
Sharding: 2 heads per core. Each core computes qkv projection for its head
columns, full causal attention for its heads, and a partial output
projection (its W_proj rows); partials are summed on host.

v2: all matmuls in bf16 (fp32r ran ~80% over bf16 speed at N=512 on HW),
softmax exp writes bf16 directly, the softmax denominator is replicated
across 64 PSUM partitions by 64 ones-columns in the AV stationary (one
reciprocal + one multiply per head per q-tile, no scalar-engine copies or
gpsimd broadcasts), and the output projection is DMA'd straight from PSUM
to DRAM with no SBUF bounce.
"""

import numpy as np
from ml_dtypes import bfloat16

import concourse.bass as bass
import concourse.mybir as mybir
from concourse import bacc
from concourse.tile import TileContext
from concourse.masks import make_identity

# Note: walrus's --enable-ldw-opt=true rejects the row-tiled bf16
# LDWEIGHTS this kernel emits ("InstLdweights is not compatible with LDW
# optimization"), so unlike the fp32r variant we leave it at the default.

F32 = mybir.dt.float32
BF16 = mybir.dt.bfloat16

B, T, C, H = 4, 2048, 1024, 16
HD = 64
NCORES = 8
CT = C // 128          # 8 C-tiles (contraction)
QT = 512               # q tile (free dim of S^T matmuls)
KT = 128               # k tile (partition dim of S^T)
VW = 256               # v_sb columns per 128-token subtile: [vA|1s|vB|1s]
SCALE = 1.0 / np.sqrt(HD)

_CACHED = {}


def build_kernel(b=B, t=T, debug_dump=False):
    """Build the per-core SPMD program. t must be a multiple of 512."""
    assert t % QT == 0
    nq = t // QT           # q-tiles per sequence
    nst = t // 128         # 128-token subtiles per sequence
    bt = b * t

    nc = bacc.Bacc("TRN2", target_bir_lowering=False, debug=False,
                   num_devices=NCORES)
    dbg = {}
    if debug_dump:
        dbg["qT"] = nc.dram_tensor("dbg_qT", [128, t], BF16,
                                   kind="ExternalOutput")
        dbg["kT"] = nc.dram_tensor("dbg_kT", [128, t], BF16,
                                   kind="ExternalOutput")
        dbg["v"] = nc.dram_tensor("dbg_v", [128, (t // 128) * VW], BF16,
                                  kind="ExternalOutput")
        dbg["yT"] = nc.dram_tensor("dbg_yT", [128, t], BF16,
                                   kind="ExternalOutput")
        dbg["es"] = nc.dram_tensor("dbg_es", [128, 2 * QT], BF16,
                                   kind="ExternalOutput")
        dbg["rr"] = nc.dram_tensor("dbg_rr", [64, QT], F32,
                                   kind="ExternalOutput")
        dbg["ya"] = nc.dram_tensor("dbg_ya", [128, QT], F32,
                                   kind="ExternalOutput")
        dbg["bc"] = nc.dram_tensor("dbg_bc", [64, QT], F32,
                                   kind="ExternalOutput")

    xT = nc.dram_tensor("xT", [C, bt], BF16, kind="ExternalInput")
    wq = nc.dram_tensor("wq", [C, 128], BF16, kind="ExternalInput")
    wk = nc.dram_tensor("wk", [C, 128], BF16, kind="ExternalInput")
    wv = nc.dram_tensor("wv", [C, 128], BF16, kind="ExternalInput")
    wp = nc.dram_tensor("wp", [128, C], BF16, kind="ExternalInput")
    bq = nc.dram_tensor("bq", [128, 1], F32, kind="ExternalInput")
    bk = nc.dram_tensor("bk", [128, 1], F32, kind="ExternalInput")
    out = nc.dram_tensor("out", [bt, C], BF16, kind="ExternalOutput")

    with TileContext(nc) as tc:
        with (
            tc.tile_pool(name="const", bufs=1) as constp,
            tc.tile_pool(name="xin", bufs=2 * CT) as xin,
            tc.tile_pool(name="qk", bufs=2) as qkp,
            tc.tile_pool(name="yt", bufs=2) as ytp,
            tc.tile_pool(name="es", bufs=3) as esp,
            tc.tile_pool(name="small", bufs=4) as smallp,
            tc.tile_pool(name="outsb", bufs=3) as outp,
            tc.tile_pool(name="ps_s", bufs=2, space="PSUM") as ps_s,
            tc.tile_pool(name="ps_ya", bufs=2, space="PSUM") as ps_ya,
            tc.tile_pool(name="ps_misc", bufs=2, space="PSUM") as ps_misc,
        ):
            # ---- constants / weights ----
            ident = constp.tile([128, 128], F32, tag="ident")
            make_identity(nc, ident[:])
            wq_sb = constp.tile([128, C], BF16, tag="wq")
            wk_sb = constp.tile([128, C], BF16, tag="wk")
            wv_sb = constp.tile([128, C], BF16, tag="wv")
            wp_sb = constp.tile([128, C], BF16, tag="wp")
            # lhsT layout: [p, ct*128 + m] = W[ct*128 + p, m]
            for w_dram, w_sb in ((wq, wq_sb), (wk, wk_sb), (wv, wv_sb)):
                nc.gpsimd.dma_start(
                    out=w_sb[:].rearrange("p (ct m) -> p ct m", ct=CT),
                    in_=w_dram[:].rearrange("(ct p) m -> p ct m", p=128),
                )
            nc.gpsimd.dma_start(out=wp_sb[:], in_=wp[:])
            bq_sb = constp.tile([128, 1], F32, tag="bq")
            bk_sb = constp.tile([128, 1], F32, tag="bk")
            nc.sync.dma_start(out=bq_sb[:], in_=bq[:])
            nc.sync.dma_start(out=bk_sb[:], in_=bk[:])
            one_bf = constp.tile([128, 1], BF16, tag="one")
            nc.vector.memset(one_bf[:], 1.0)

            xts_all = {}

            def load_x(bi):
                tiles = [xin.tile([128, t], BF16, tag="xt",
                                  name=f"xt{bi}_{ct}")
                         for ct in range(CT)]
                for ct in range(CT):
                    nc.gpsimd.dma_start(
                        out=tiles[ct][:],
                        in_=xT[ct * 128:(ct + 1) * 128,
                               bi * t:(bi + 1) * t])
                xts_all[bi] = tiles

            load_x(0)
            for bi in range(b):
                xts = xts_all[bi]
                # v_sb per 128-token subtile: [vA(64)|1s(64)|vB(64)|1s(64)].
                # The ones columns replicate the softmax denominator across
                # PSUM partitions 64..127 of the AV output.
                v_sb = qkp.tile([128, nst * VW], BF16, tag="v")
                v_hw = v_sb[:].rearrange("p (g w) -> p g w", w=128)
                nc.gpsimd.memset(v_hw[:, :, 64:128], 1.0)
                qT_sb = qkp.tile([128, t], BF16, tag="qT")
                kT_sb = qkp.tile([128, t], BF16, tag="kT")
                yT_sb = ytp.tile([128, t], BF16, tag="yT")

                # ================= QKV projection =================
                for colt in range(t // QT):
                    csl = slice(colt * QT, (colt + 1) * QT)
                    for w_sb, dst, bias in (
                        (wk_sb, kT_sb, bk_sb), (wq_sb, qT_sb, bq_sb),
                    ):
                        ps = ps_misc.tile([128, QT], F32, tag="m")
                        for ct in range(CT):
                            nc.tensor.matmul(
                                ps[:],
                                w_sb[:, ct * 128:(ct + 1) * 128],
                                xts[ct][:, csl],
                                start=(ct == 0), stop=(ct == CT - 1),
                            )
                        nc.vector.tensor_scalar_add(
                            out=dst[:, csl], in0=ps[:], scalar1=bias[:])
                    # V^T for this col tile, then transpose to natural layout
                    ps = ps_misc.tile([128, QT], F32, tag="m")
                    for ct in range(CT):
                        nc.tensor.matmul(
                            ps[:], wv_sb[:, ct * 128:(ct + 1) * 128],
                            xts[ct][:, csl],
                            start=(ct == 0), stop=(ct == CT - 1))
                    vt_col = smallp.tile([128, QT], F32, tag="vtcol")
                    nc.vector.tensor_copy(out=vt_col[:], in_=ps[:])
                    tp = ps_misc.tile([128, QT], F32, tag="m")
                    for sj in range(QT // 128):
                        nc.tensor.transpose(
                            tp[:, sj * 128:(sj + 1) * 128],
                            vt_col[:, sj * 128:(sj + 1) * 128],
                            ident[:])
                    for sj in range(QT // 128):
                        st = colt * (QT // 128) + sj
                        src = tp[:, sj * 128:(sj + 1) * 128].rearrange(
                            "p (h w) -> p h w", h=2)
                        dstv = v_sb[:, st * VW:(st + 1) * VW].rearrange(
                            "p (h w) -> p h w", h=2)[:, :, 0:64]
                        nc.vector.tensor_copy(out=dstv, in_=src)

                if bi + 1 < b:
                    load_x(bi + 1)
                if debug_dump and bi == 0:
                    nc.sync.dma_start(out=dbg["qT"][:], in_=qT_sb[:])
                    nc.sync.dma_start(out=dbg["kT"][:], in_=kT_sb[:])
                    nc.sync.dma_start(out=dbg["v"][:], in_=v_sb[:])

                # ================= attention (heads row-paired) ==========
                for qt in range(nq):
                    n_k = (qt + 1) * (QT // KT)   # k-tiles of 128
                    q0 = qt * QT
                    yas = [ps_ya.tile([128, QT], F32, tag="ya",
                                      name=f"ya{_h}")
                           for _h in range(2)]
                    for kt in range(n_k):
                        lo = max(0, kt * KT - q0)
                        # S^T for both heads in one array pass: head A on PE
                        # rows 0-63, head B on rows 64-127 (row tiling).
                        sg = ps_s.tile([128, 2 * QT], F32, tag="sg")
                        es = esp.tile([128, 2 * QT], BF16, tag="es")
                        for h in range(2):
                            hsl = slice(h * 64, (h + 1) * 64)
                            nc.tensor.matmul(
                                sg[:, h * QT + lo:(h + 1) * QT],
                                kT_sb[hsl, kt * KT:(kt + 1) * KT],
                                qT_sb[hsl, q0 + lo:q0 + QT],
                                start=True, stop=True,
                            )
                        # exp for both heads in one op; on diagonal tiles
                        # only the causally-reachable cols [lo:] are computed
                        sg_v = sg[:].rearrange("p (h q) -> p h q", h=2)
                        es_v = es[:].rearrange("p (h q) -> p h q", h=2)
                        nc.scalar.activation(
                            es_v[:, :, lo:], sg_v[:, :, lo:],
                            mybir.ActivationFunctionType.Exp, scale=SCALE)
                        if kt * KT >= q0:
                            # causal band select, both heads in one op
                            nc.gpsimd.affine_select(
                                out=es_v[:, :, lo:lo + KT],
                                in_=es_v[:, :, lo:lo + KT],
                                compare_op=mybir.AluOpType.is_ge,
                                fill=0.0,
                                base=0,
                                channel_multiplier=-1,
                                pattern=[[0, 2], [1, KT]],
                            )
                        if debug_dump and bi == 0 and qt == 0 and kt == 0:
                            nc.sync.dma_start(out=dbg["es"][:], in_=es[:])
                        for h in range(2):
                            nc.tensor.matmul(
                                yas[h][:, lo:QT],
                                v_sb[:, kt * VW + h * 128:
                                     kt * VW + (h + 1) * 128],
                                es[:, h * QT + lo:(h + 1) * QT],
                                start=(kt == 0), stop=(kt == n_k - 1),
                            )
                    for h in range(2):
                        ya = yas[h]
                        # One full-tile PSUM->SBUF copy frees the ya bank
                        # immediately (the next q-tile's AV matmuls need it);
                        # the denominator chain then runs from SBUF:
                        # DMA-shift den down to partitions 0-63, reciprocal,
                        # fused multiply.
                        ya_sb = smallp.tile([128, QT], F32, tag="yasb")
                        nc.vector.tensor_copy(out=ya_sb[:], in_=ya[:])
                        dnlo = smallp.tile([64, QT], F32, tag="dnlo")
                        nc.sync.dma_start(out=dnlo[:], in_=ya_sb[64:128, :])
                        bc = smallp.tile([64, QT], F32, tag="bc")
                        nc.vector.reciprocal_approx_fast(
                            out=bc[:], in_=dnlo[:])
                        if debug_dump and bi == 0 and qt == 0 and h == 0:
                            nc.sync.dma_start(out=dbg["rr"][:], in_=bc[:])
                            nc.sync.dma_start(out=dbg["bc"][:], in_=bc[:])
                            nc.sync.dma_start(out=dbg["ya"][:], in_=ya_sb[:])
                        if h == 0:
                            nc.vector.tensor_mul(
                                out=yT_sb[0:64, q0:q0 + QT],
                                in0=ya_sb[0:64, :], in1=bc[:])
                        else:
                            ytb = smallp.tile([64, QT], BF16, tag="ytb")
                            nc.vector.tensor_mul(
                                out=ytb[:], in0=ya_sb[0:64, :], in1=bc[:])
                            nc.sync.dma_start(
                                out=yT_sb[64:128, q0:q0 + QT], in_=ytb[:])

                    # ======== output projection for this q-tile ========
                    for sj in range(QT // 128):
                        st = qt * (QT // 128) + sj
                        osb = outp.tile([128, C], BF16, tag="osb")
                        for n in range(C // QT):
                            pp = ps_misc.tile([128, QT], F32, tag="m")
                            nc.tensor.matmul(
                                pp[:],
                                yT_sb[:, st * 128:(st + 1) * 128],
                                wp_sb[:, n * QT:(n + 1) * QT],
                                start=True, stop=True)
                            if n == 0:
                                nc.vector.tensor_copy(
                                    out=osb[:, n * QT:(n + 1) * QT],
                                    in_=pp[:])
                            else:
                                nc.scalar.copy(
                                    out=osb[:, n * QT:(n + 1) * QT],
                                    in_=pp[:])
                        nc.gpsimd.dma_start(
                            out=out[bi * t + st * 128:
                                    bi * t + (st + 1) * 128, :],
                            in_=osb[:])
                if debug_dump and bi == 0:
                    nc.sync.dma_start(out=dbg["yT"][:], in_=yT_sb[:])

    nc.compile()
    return nc


def _prep_inputs(x, W_attn, b_attn, W_proj, b_proj, b, t):
    xT_full = np.ascontiguousarray(
        x.reshape(b * t, C).T).astype(bfloat16)
    in_maps = []
    for c in range(NCORES):
        sl = slice(c * 128, (c + 1) * 128)
        in_maps.append({
            "xT": xT_full,
            "wq": np.ascontiguousarray(W_attn[:, sl]).astype(bfloat16),
            "wk": np.ascontiguousarray(
                W_attn[:, 1024:2048][:, sl]).astype(bfloat16),
            "wv": np.ascontiguousarray(
                W_attn[:, 2048:3072][:, sl]).astype(bfloat16),
            "wp": np.ascontiguousarray(W_proj[sl, :]).astype(bfloat16),
            "bq": np.ascontiguousarray(b_attn[sl].reshape(128, 1)),
            "bk": np.ascontiguousarray(b_attn[1024:2048][sl].reshape(128, 1)),
        })
    return in_maps


def kernel(x, W_attn, b_attn, W_proj, b_proj, _trace=False):
    from concourse.bass_utils import run_bass_kernel_spmd

    x = np.asarray(x, dtype=np.float32)
    W_attn = np.asarray(W_attn, dtype=np.float32)
    b_attn = np.asarray(b_attn, dtype=np.float32)
    W_proj = np.asarray(W_proj, dtype=np.float32)
    b_proj = np.asarray(b_proj, dtype=np.float32)
    b, t, c = x.shape

    key = (b, t)
    if key not in _CACHED:
        _CACHED[key] = build_kernel(b, t)
    nc = _CACHED[key]

    in_maps = _prep_inputs(x, W_attn, b_attn, W_proj, b_proj, b, t)
    res = run_bass_kernel_spmd(
        nc, in_maps, core_ids=list(range(NCORES)), trace=_trace)

    acc = res.results[0]["out"].astype(np.float32).copy()
    for r in res.results[1:]:
        acc += r["out"]
    acc += b_attn[2048:3072] @ W_proj + b_proj
    out = acc.reshape(b, t, c)
    if _trace:
        kernel.last_result = res
    return out


# revision 27
# speedup vs baseline: 1.2707x; 1.1785x over previous
"""Causal self-attention Trainium2 kernel — tensor-parallel over heads on 8 NeuronCores.

Problem: B=4, T=2048, C=1024, H=16 heads (head_dim 64), fp32 in/out.
Sharding: 2 heads per core. Each core computes qkv projection for its head
columns, full causal attention for its heads, and a partial output
projection (its W_proj rows); partials are summed on host.

v2: all matmuls in bf16 (fp32r ran ~80% over bf16 speed at N=512 on HW),
softmax exp writes bf16 directly, the softmax denominator is replicated
across 64 PSUM partitions by 64 ones-columns in the AV stationary (one
reciprocal + one multiply per head per q-tile, no scalar-engine copies or
gpsimd broadcasts), and the output projection is DMA'd straight from PSUM
to DRAM with no SBUF bounce.
"""

import numpy as np
from ml_dtypes import bfloat16

import concourse.bass as bass
import concourse.mybir as mybir
from concourse import bacc
from concourse.tile import TileContext
from concourse.masks import make_identity

# Note: walrus's --enable-ldw-opt=true rejects the row-tiled bf16
# LDWEIGHTS this kernel emits ("InstLdweights is not compatible with LDW
# optimization"), so unlike the fp32r variant we leave it at the default.

F32 = mybir.dt.float32
BF16 = mybir.dt.bfloat16

B, T, C, H = 4, 2048, 1024, 16
HD = 64
NCORES = 8
CT = C // 128          # 8 C-tiles (contraction)
QT = 512               # q tile (free dim of S^T matmuls)
KT = 128               # k tile (partition dim of S^T)
VW = 256               # v_sb columns per 128-token subtile: [vA|1s|vB|1s]
SCALE = 1.0 / np.sqrt(HD)

_CACHED = {}


def build_kernel(b=B, t=T, debug_dump=False):
    """Build the per-core SPMD program. t must be a multiple of 512."""
    assert t % QT == 0
    nq = t // QT           # q-tiles per sequence
    nst = t // 128         # 128-token subtiles per sequence
    bt = b * t

    nc = bacc.Bacc("TRN2", target_bir_lowering=False, debug=False,
                   num_devices=NCORES)
    dbg = {}
    if debug_dump:
        dbg["qT"] = nc.dram_tensor("dbg_qT", [128, t], BF16,
                                   kind="ExternalOutput")
        dbg["kT"] = nc.dram_tensor("dbg_kT", [128, t], BF16,
                                   kind="ExternalOutput")
        dbg["v"] = nc.dram_tensor("dbg_v", [128, (t // 128) * VW], BF16,
                                  kind="ExternalOutput")
        dbg["yT"] = nc.dram_tensor("dbg_yT", [128, t], BF16,
                                   kind="ExternalOutput")
        dbg["es"] = nc.dram_tensor("dbg_es", [128, 2 * QT], BF16,
                                   kind="ExternalOutput")
        dbg["rr"] = nc.dram_tensor("dbg_rr", [64, QT], F32,
                                   kind="ExternalOutput")
        dbg["ya"] = nc.dram_tensor("dbg_ya", [128, QT], F32,
                                   kind="ExternalOutput")
        dbg["bc"] = nc.dram_tensor("dbg_bc", [64, QT], F32,
                                   kind="ExternalOutput")

    xT = nc.dram_tensor("xT", [C, bt], BF16, kind="ExternalInput")
    wq = nc.dram_tensor("wq", [C, 128], BF16, kind="ExternalInput")
    wk = nc.dram_tensor("wk", [C, 128], BF16, kind="ExternalInput")
    wv = nc.dram_tensor("wv", [C, 128], BF16, kind="ExternalInput")
    wp = nc.dram_tensor("wp", [128, C], BF16, kind="ExternalInput")
    bq = nc.dram_tensor("bq", [128, 1], F32, kind="ExternalInput")
    bk = nc.dram_tensor("bk", [128, 1], F32, kind="ExternalInput")
    out = nc.dram_tensor("out", [bt, C], BF16, kind="ExternalOutput")

    with TileContext(nc) as tc:
        with (
            tc.tile_pool(name="const", bufs=1) as constp,
            tc.tile_pool(name="xin", bufs=2 * CT) as xin,
            tc.tile_pool(name="qk", bufs=2) as qkp,
            tc.tile_pool(name="yt", bufs=2) as ytp,
            tc.tile_pool(name="es", bufs=3) as esp,
            tc.tile_pool(name="small", bufs=4) as smallp,
            tc.tile_pool(name="outsb", bufs=3) as outp,
            tc.tile_pool(name="ps_s", bufs=2, space="PSUM") as ps_s,
            tc.tile_pool(name="ps_ya", bufs=2, space="PSUM") as ps_ya,
            tc.tile_pool(name="ps_misc", bufs=2, space="PSUM") as ps_misc,
        ):
            # ---- constants / weights ----
            ident = constp.tile([128, 128], F32, tag="ident")
            make_identity(nc, ident[:])
            wq_sb = constp.tile([128, C], BF16, tag="wq")
            wk_sb = constp.tile([128, C], BF16, tag="wk")
            wv_sb = constp.tile([128, C], BF16, tag="wv")
            wp_sb = constp.tile([128, C], BF16, tag="wp")
            # lhsT layout: [p, ct*128 + m] = W[ct*128 + p, m]
            for w_dram, w_sb in ((wq, wq_sb), (wk, wk_sb), (wv, wv_sb)):
                nc.gpsimd.dma_start(
                    out=w_sb[:].rearrange("p (ct m) -> p ct m", ct=CT),
                    in_=w_dram[:].rearrange("(ct p) m -> p ct m", p=128),
                )
            nc.gpsimd.dma_start(out=wp_sb[:], in_=wp[:])
            bq_sb = constp.tile([128, 1], F32, tag="bq")
            bk_sb = constp.tile([128, 1], F32, tag="bk")
            nc.sync.dma_start(out=bq_sb[:], in_=bq[:])
            nc.sync.dma_start(out=bk_sb[:], in_=bk[:])
            one_bf = constp.tile([128, 1], BF16, tag="one")
            nc.vector.memset(one_bf[:], 1.0)

            xts_all = {}

            def load_x(bi):
                tiles = [xin.tile([128, t], BF16, tag="xt",
                                  name=f"xt{bi}_{ct}")
                         for ct in range(CT)]
                for ct in range(CT):
                    nc.gpsimd.dma_start(
                        out=tiles[ct][:],
                        in_=xT[ct * 128:(ct + 1) * 128,
                               bi * t:(bi + 1) * t])
                xts_all[bi] = tiles

            # Deferred projection steps: each closure does one 128-token
            # block of y @ W_proj (+ evacuation + output DMA). They are
            # popped one per attention kt-iteration so the projection
            # matmuls fill the exp-paced gaps in the PE queue.
            proj_queue = []

            def mk_proj(pbi, yT_tile, st, sj):
                def go():
                    osb = outp.tile([128, C], BF16, tag="osb",
                                    name=f"osb{pbi}_{st}")
                    for n in range(C // QT):
                        pp = ps_misc.tile([128, QT], F32, tag="m",
                                          name=f"pp{pbi}_{st}_{n}")
                        nc.tensor.matmul(
                            pp[:],
                            yT_tile[:, st * 128:(st + 1) * 128],
                            wp_sb[:, n * QT:(n + 1) * QT],
                            start=True, stop=True)
                        if n == 1 and sj % 2 == 0:
                            nc.scalar.copy(
                                out=osb[:, n * QT:(n + 1) * QT], in_=pp[:])
                        else:
                            nc.vector.tensor_copy(
                                out=osb[:, n * QT:(n + 1) * QT], in_=pp[:])
                    nc.gpsimd.dma_start(
                        out=out[pbi * t + st * 128:
                                pbi * t + (st + 1) * 128, :],
                        in_=osb[:])
                return go

            load_x(0)
            for bi in range(b):
                xts = xts_all[bi]
                # v_sb per 128-token subtile: [vA(64)|1s(64)|vB(64)|1s(64)].
                # The ones columns replicate the softmax denominator across
                # PSUM partitions 64..127 of the AV output.
                v_sb = qkp.tile([128, nst * VW], BF16, tag="v")
                v_hw = v_sb[:].rearrange("p (g w) -> p g w", w=128)
                nc.gpsimd.memset(v_hw[:, :, 64:128], 1.0)
                qT_sb = qkp.tile([128, t], BF16, tag="qT")
                kT_sb = qkp.tile([128, t], BF16, tag="kT")
                yT_sb = ytp.tile([128, t], BF16, tag="yT")

                # ================= QKV projection =================
                for colt in range(t // QT):
                    csl = slice(colt * QT, (colt + 1) * QT)
                    for w_sb, dst, bias in (
                        (wk_sb, kT_sb, bk_sb), (wq_sb, qT_sb, bq_sb),
                    ):
                        ps = ps_misc.tile([128, QT], F32, tag="m")
                        for ct in range(CT):
                            nc.tensor.matmul(
                                ps[:],
                                w_sb[:, ct * 128:(ct + 1) * 128],
                                xts[ct][:, csl],
                                start=(ct == 0), stop=(ct == CT - 1),
                            )
                        nc.vector.tensor_scalar_add(
                            out=dst[:, csl], in0=ps[:], scalar1=bias[:])
                    # V^T for this col tile, then transpose to natural layout
                    ps = ps_misc.tile([128, QT], F32, tag="m")
                    for ct in range(CT):
                        nc.tensor.matmul(
                            ps[:], wv_sb[:, ct * 128:(ct + 1) * 128],
                            xts[ct][:, csl],
                            start=(ct == 0), stop=(ct == CT - 1))
                    vt_col = smallp.tile([128, QT], F32, tag="vtcol")
                    nc.vector.tensor_copy(out=vt_col[:], in_=ps[:])
                    tp = ps_misc.tile([128, QT], F32, tag="m")
                    for sj in range(QT // 128):
                        nc.tensor.transpose(
                            tp[:, sj * 128:(sj + 1) * 128],
                            vt_col[:, sj * 128:(sj + 1) * 128],
                            ident[:])
                    for sj in range(QT // 128):
                        st = colt * (QT // 128) + sj
                        src = tp[:, sj * 128:(sj + 1) * 128].rearrange(
                            "p (h w) -> p h w", h=2)
                        dstv = v_sb[:, st * VW:(st + 1) * VW].rearrange(
                            "p (h w) -> p h w", h=2)[:, :, 0:64]
                        nc.vector.tensor_copy(out=dstv, in_=src)

                if bi + 1 < b:
                    load_x(bi + 1)
                if debug_dump and bi == 0:
                    nc.sync.dma_start(out=dbg["qT"][:], in_=qT_sb[:])
                    nc.sync.dma_start(out=dbg["kT"][:], in_=kT_sb[:])
                    nc.sync.dma_start(out=dbg["v"][:], in_=v_sb[:])

                # ============ attention (heads row-paired) ============
                # Software-pipelined: S^T+exp for unit i+1 is emitted
                # before AV of unit i so the PE never queues behind the
                # exp; one deferred projection step is woven into each kt
                # iteration to fill the exp-paced gaps.
                units = [(qt, kt) for qt in range(nq)
                         for kt in range((qt + 1) * (QT // KT))]
                st_state = {}

                def emit_st(qt, kt):
                    q0 = qt * QT
                    lo = max(0, kt * KT - q0)
                    sg = ps_s.tile([128, 2 * QT], F32, tag="sg",
                                   name=f"sg{bi}_{qt}_{kt}")
                    es = esp.tile([128, 2 * QT], BF16, tag="es",
                                  name=f"es{bi}_{qt}_{kt}")
                    for h in range(2):
                        hsl = slice(h * 64, (h + 1) * 64)
                        nc.tensor.matmul(
                            sg[:, h * QT + lo:(h + 1) * QT],
                            kT_sb[hsl, kt * KT:(kt + 1) * KT],
                            qT_sb[hsl, q0 + lo:q0 + QT],
                            start=True, stop=True,
                        )
                    # exp for both heads in one op; on diagonal tiles only
                    # the causally-reachable cols [lo:] are computed
                    sg_v = sg[:].rearrange("p (h q) -> p h q", h=2)
                    es_v = es[:].rearrange("p (h q) -> p h q", h=2)
                    nc.scalar.activation(
                        es_v[:, :, lo:], sg_v[:, :, lo:],
                        mybir.ActivationFunctionType.Exp, scale=SCALE)
                    if kt * KT >= q0:
                        # causal band select, both heads in one op
                        nc.gpsimd.affine_select(
                            out=es_v[:, :, lo:lo + KT],
                            in_=es_v[:, :, lo:lo + KT],
                            compare_op=mybir.AluOpType.is_ge,
                            fill=0.0,
                            base=0,
                            channel_multiplier=-1,
                            pattern=[[0, 2], [1, KT]],
                        )
                    if debug_dump and bi == 0 and qt == 0 and kt == 0:
                        nc.sync.dma_start(out=dbg["es"][:], in_=es[:])
                    st_state[(qt, kt)] = (es, lo)

                emit_st(*units[0])
                yas = None
                for idx, (qt, kt) in enumerate(units):
                    n_k = (qt + 1) * (QT // KT)
                    q0 = qt * QT
                    if kt == 0:
                        yas = [ps_ya.tile([128, QT], F32, tag="ya",
                                          name=f"ya{bi}_{qt}_{_h}")
                               for _h in range(2)]
                    if idx + 1 < len(units):
                        emit_st(*units[idx + 1])
                    es, lo = st_state.pop((qt, kt))
                    for h in range(2):
                        nc.tensor.matmul(
                            yas[h][:, lo:QT],
                            v_sb[:, kt * VW + h * 128:
                                 kt * VW + (h + 1) * 128],
                            es[:, h * QT + lo:(h + 1) * QT],
                            start=(kt == 0), stop=(kt == n_k - 1),
                        )
                    if proj_queue:
                        proj_queue.pop(0)()
                    if kt != n_k - 1:
                        continue
                    # ------ end of q-tile: normalize into yT ------
                    for h in range(2):
                        ya = yas[h]
                        # One full-tile PSUM->SBUF copy frees the ya bank
                        # immediately (the next q-tile's AV matmuls need
                        # it); the denominator chain then runs from SBUF:
                        # DMA-shift den down to partitions 0-63,
                        # reciprocal, fused multiply.
                        ya_sb = smallp.tile([128, QT], F32, tag="yasb")
                        nc.vector.tensor_copy(out=ya_sb[:], in_=ya[:])
                        dnlo = smallp.tile([64, QT], F32, tag="dnlo")
                        nc.sync.dma_start(out=dnlo[:],
                                          in_=ya_sb[64:128, :])
                        bc = smallp.tile([64, QT], F32, tag="bc")
                        nc.vector.reciprocal_approx_fast(
                            out=bc[:], in_=dnlo[:])
                        if debug_dump and bi == 0 and qt == 0 and h == 0:
                            nc.sync.dma_start(out=dbg["rr"][:], in_=bc[:])
                            nc.sync.dma_start(out=dbg["bc"][:], in_=bc[:])
                            nc.sync.dma_start(out=dbg["ya"][:],
                                              in_=ya_sb[:])
                        if h == 0:
                            nc.vector.tensor_mul(
                                out=yT_sb[0:64, q0:q0 + QT],
                                in0=ya_sb[0:64, :], in1=bc[:])
                        else:
                            ytb = smallp.tile([64, QT], BF16, tag="ytb")
                            nc.vector.tensor_mul(
                                out=ytb[:], in0=ya_sb[0:64, :], in1=bc[:])
                            nc.sync.dma_start(
                                out=yT_sb[64:128, q0:q0 + QT], in_=ytb[:])
                    # queue this q-tile's projection for interleaving
                    for sj in range(QT // 128):
                        proj_queue.append(
                            mk_proj(bi, yT_sb,
                                    qt * (QT // 128) + sj, sj))
                if debug_dump and bi == 0:
                    nc.sync.dma_start(out=dbg["yT"][:], in_=yT_sb[:])

            # drain any remaining projection steps (last q-tiles)
            while proj_queue:
                proj_queue.pop(0)()

    nc.compile()
    return nc


def _prep_inputs(x, W_attn, b_attn, W_proj, b_proj, b, t):
    xT_full = np.ascontiguousarray(
        x.reshape(b * t, C).T).astype(bfloat16)
    in_maps = []
    for c in range(NCORES):
        sl = slice(c * 128, (c + 1) * 128)
        in_maps.append({
            "xT": xT_full,
            "wq": np.ascontiguousarray(W_attn[:, sl]).astype(bfloat16),
            "wk": np.ascontiguousarray(
                W_attn[:, 1024:2048][:, sl]).astype(bfloat16),
            "wv": np.ascontiguousarray(
                W_attn[:, 2048:3072][:, sl]).astype(bfloat16),
            "wp": np.ascontiguousarray(W_proj[sl, :]).astype(bfloat16),
            "bq": np.ascontiguousarray(b_attn[sl].reshape(128, 1)),
            "bk": np.ascontiguousarray(b_attn[1024:2048][sl].reshape(128, 1)),
        })
    return in_maps


def kernel(x, W_attn, b_attn, W_proj, b_proj, _trace=False):
    from concourse.bass_utils import run_bass_kernel_spmd

    x = np.asarray(x, dtype=np.float32)
    W_attn = np.asarray(W_attn, dtype=np.float32)
    b_attn = np.asarray(b_attn, dtype=np.float32)
    W_proj = np.asarray(W_proj, dtype=np.float32)
    b_proj = np.asarray(b_proj, dtype=np.float32)
    b, t, c = x.shape

    key = (b, t)
    if key not in _CACHED:
        _CACHED[key] = build_kernel(b, t)
    nc = _CACHED[key]

    in_maps = _prep_inputs(x, W_attn, b_attn, W_proj, b_proj, b, t)
    res = run_bass_kernel_spmd(
        nc, in_maps, core_ids=list(range(NCORES)), trace=_trace)

    acc = res.results[0]["out"].astype(np.float32).copy()
    for r in res.results[1:]:
        acc += r["out"]
    acc += b_attn[2048:3072] @ W_proj + b_proj
    out = acc.reshape(b, t, c)
    if _trace:
        kernel.last_result = res
    return out
